# revision 1
# baseline (speedup 1.0000x reference)
"""Trainium2 Bass kernel for nn_GeneralizedKernelScore (loss_fn).

Math per sample n (M=8 population members, D=12288 features):
    beta      = 2.0 - 1.9*t/999                      (linear schedule from t)
    conf[n]   = mean_j    exp(-beta*||x_j - y_j||^2 / D)
    inter[n]  = mean_{j!=j'} exp(-beta*||x_j - x_j'||^2 / D)
    im[n]     = inter/2
    score[n]  = im - conf

Strategy (data-parallel over batch, 4 samples per core on 8 cores):
Each core owns Z = [X; Y] (64 rows x 12288).  Every distance the loss
needs comes from the 64x64 Gram matrix G = Z Z^T:
    ||z_a - z_b||^2 = G[a,a] + G[b,b] - 2 G[a,b]
G is computed as 96 accumulating bf16 matmuls over 128-wide slices of
the feature dim.  The host pre-transposes each core's shard into
feature-major layout [128, 96*64] so every DMA byte is contiguous and
the contraction dim lands on SBUF partitions with no on-device
transpose.  The small post-processing (norm extraction via masked
reduce, exp on ScalarE with fused accumulate, per-sample partition sums
via tiny 0/1 selection matmuls) is all on-device.
"""

from contextlib import ExitStack

import numpy as np

import concourse.bass as bass
import concourse.mybir as mybir
import concourse.tile as tile
from concourse import bacc
from concourse.bass_utils import run_bass_kernel_spmd

# problem shape (hardcoded per spec)
N, M, D = 32, 8, 12288
NUM_TIMESTEPS = 1000
BETA_START, BETA_END = 2.0, 0.1
LAMBDA_VAL = 1.0

NCORES = 8
NS = N // NCORES          # 4 samples per core
R = 2 * NS * M            # 64 Z-rows per core (32 x-rows then 32 y-rows)
NCH = D // 128            # 96 contraction chunks
FREE = NCH * R            # 6144 free columns of Z^T
NDMA = 8                  # input DMA chunks
CHF = FREE // NDMA        # 768 cols per DMA chunk
CHK = NCH // NDMA         # 12 gram-chunks per DMA chunk

# sel constant column layout
_A0, _J0, _P0, _R0, _S0, _B0 = 0, 32, 40, 44, 76, 108
SELW = 140

# how Z^T reaches bf16 SBUF: "dve" = fp32 HWDGE DMA + DVE cast,
# "dma" = SWDGE cast-during-DMA, "bf16" = host sends bf16 over HBM.
CAST_MODE = "bf16"

F32 = mybir.dt.float32
BF16 = mybir.dt.bfloat16


def _build_consts():
    p = np.arange(128)[:, None]
    i32 = np.arange(32)[None, :]
    A = ((p // 8) == (i32 // 8)) & (p < 32)            # [128,32] sample-block
    J8 = ((p % 8) == np.arange(8)[None, :]) & (p < 32)  # [128,8]
    P4 = ((p // 8) == np.arange(4)[None, :]) & (p < 32)  # [128,4]
    R4 = p == (i32 // 8)                                # [128,32] beta spread
    SH = p == (i32 + 32)                                # [128,32] y-row shift
    # block-diagonal mask with the -2 distance coefficient folded in
    BLK = np.where(((p // 8) == (i32 // 8)) & (p < 32), -2.0, 0.0)
    sel = np.concatenate([A, J8, P4, R4, SH, BLK], axis=1).astype(np.float32)
    i64 = np.eye(64, dtype=np.float32)
    return sel, i64


def _build_program(cast_mode=CAST_MODE):
    nc = bacc.Bacc("TRN2", target_bir_lowering=False)
    zt_dt = BF16 if cast_mode == "bf16" else F32
    zt = nc.dram_tensor("zt", [128, FREE], zt_dt, kind="ExternalInput")
    tq = nc.dram_tensor("tq", [NS, 1], mybir.dt.int32, kind="ExternalInput")
    sel_d = nc.dram_tensor("sel", [128, SELW], F32, kind="ExternalInput")
    i64_d = nc.dram_tensor("i64", [64, 64], F32, kind="ExternalInput")
    res_d = nc.dram_tensor("res", [NS, 4], F32, kind="ExternalOutput")

    add, mult, sub = (
        mybir.AluOpType.add,
        mybir.AluOpType.mult,
        mybir.AluOpType.subtract,
    )
    EXP = mybir.ActivationFunctionType.Exp

    with ExitStack() as ctx:
        tc = ctx.enter_context(tile.TileContext(nc))
        small = ctx.enter_context(tc.tile_pool(name="small", bufs=1))
        zin_p = ctx.enter_context(tc.tile_pool(name="zin", bufs=NDMA))
        zbf_p = ctx.enter_context(tc.tile_pool(name="zbf", bufs=NDMA))
        psum = ctx.enter_context(tc.tile_pool(name="psum", bufs=1, space="PSUM"))

        # --- constants + t -------------------------------------------------
        sel = small.tile([128, SELW], F32, tag="sel")
        nc.sync.dma_start(out=sel, in_=sel_d[:])
        i64 = small.tile([64, 64], F32, tag="i64")
        nc.sync.dma_start(out=i64, in_=i64_d[:])
        tq_sb = small.tile([NS, 1], mybir.dt.int32, tag="tq")
        nc.sync.dma_start(out=tq_sb, in_=tq[:])

        # preload the Exp LUT while DMAs run
        warm = small.tile([1, 1], F32, tag="warm")
        nc.vector.memset(warm, 0.0)
        nc.scalar.activation(out=warm, in_=warm, func=EXP)

        # beta pipeline: bscaled[p] = -beta[n(p)]/D for the 4 local samples
        tpad = small.tile([128, 1], F32, tag="tpad")
        nc.vector.memset(tpad, 0.0)
        nc.vector.tensor_copy(out=tpad[0:NS, :], in_=tq_sb)  # int32 -> f32
        bsc = small.tile([128, 1], F32, tag="bsc")
        nc.vector.memset(bsc, 0.0)
        # -beta/D = (1.9/999)*t/D - 2.0/D
        nc.vector.tensor_scalar(
            out=bsc[0:NS, :],
            in0=tpad[0:NS, :],
            scalar1=(BETA_START - BETA_END) / ((NUM_TIMESTEPS - 1) * D),
            scalar2=-BETA_START / D,
            op0=mult,
            op1=add,
        )
        bvp = psum.tile([32, 1], F32, tag="bvp")
        nc.tensor.matmul(bvp, lhsT=sel[:, _R0 : _R0 + 32], rhs=bsc, start=True, stop=True)
        bvec = small.tile([32, 1], F32, tag="bvec")
        nc.vector.tensor_copy(out=bvec, in_=bvp)

        # zero-init tiles used as padded matmul operands later
        xn = small.tile([128, 1], F32, tag="xn")
        nc.vector.memset(xn, 0.0)
        sc = small.tile([128, 2], F32, tag="sc")
        nc.vector.memset(sc, 0.0)

        # --- the Gram matrix G = Z Z^T (64x64, fp32 in PSUM) ---------------
        G = psum.tile([R, R], F32, tag="G")
        for i in range(NDMA):
            if cast_mode == "bf16":
                zbf = zbf_p.tile([128, CHF], BF16, tag="zbf")
                nc.sync.dma_start(out=zbf, in_=zt[:, i * CHF : (i + 1) * CHF])
            elif cast_mode == "dma":
                zbf = zbf_p.tile([128, CHF], BF16, tag="zbf")
                nc.gpsimd.dma_start(out=zbf, in_=zt[:, i * CHF : (i + 1) * CHF])
            else:  # "dve"
                zin = zin_p.tile([128, CHF], F32, tag="zin")
                nc.sync.dma_start(out=zin, in_=zt[:, i * CHF : (i + 1) * CHF])
                zbf = zbf_p.tile([128, CHF], BF16, tag="zbf")
                nc.vector.tensor_copy(out=zbf, in_=zin)
            for j in range(CHK):
                k = i * CHK + j
                sl = zbf[:, j * R : (j + 1) * R]
                nc.tensor.matmul(
                    G, lhsT=sl, rhs=sl, start=(k == 0), stop=(k == NCH - 1)
                )

        # --- post-processing ----------------------------------------------
        # norms of all 64 Z rows: diag(G), via masked multiply + reduce
        # (tensor_tensor_reduce would fuse these but faults on this runtime)
        s64 = small.tile([64, 64], F32, tag="s64")
        nc.vector.tensor_tensor(out=s64, in0=G, in1=i64, op=mult)
        nc.vector.reduce_sum(
            out=xn[0:64, :], in_=s64, axis=mybir.AxisListType.X
        )
        # xy[p] = <x_p, y_p> = G[p, 32+p], p in 0..31
        s32 = small.tile([32, 32], F32, tag="s32")
        xy = small.tile([32, 1], F32, tag="xy")
        nc.vector.tensor_tensor(
            out=s32, in0=G[0:32, 32:64], in1=i64[0:32, 0:32], op=mult
        )
        nc.vector.reduce_sum(out=xy, in_=s32, axis=mybir.AxisListType.X)
        # Cm2[p, f] = -2 * <x_p, x_{n(p)*8+f}>  (per-sample 8x8 blocks):
        # mask G's x-x quadrant to its sample-diagonal blocks (mask holds
        # the -2), then compact 32 -> 8 cols with a strided group-sum.
        bmask = small.tile([32, 32], F32, tag="bmask")
        nc.vector.tensor_tensor(
            out=bmask, in0=G[0:32, 0:32], in1=sel[0:32, _B0 : _B0 + 32],
            op=mult,
        )
        cm2 = small.tile([32, 8], F32, tag="cm2")
        nc.vector.reduce_sum(
            out=cm2,
            in_=bmask[:, :].rearrange("p (g f) -> p f g", g=NS),
            axis=mybir.AxisListType.X,
        )
        # XC[p, f] = ||x_{n(p)*8+f}||^2  via selection matmul
        rhsj = small.tile([128, 8], F32, tag="rhsj")
        nc.vector.tensor_scalar(
            out=rhsj, in0=sel[:, _J0 : _J0 + 8], scalar1=xn, scalar2=None, op0=mult
        )
        xcp = psum.tile([32, 8], F32, tag="xcp")
        nc.tensor.matmul(xcp, lhsT=sel[:, _A0 : _A0 + 32], rhs=rhsj, start=True, stop=True)
        # YN[p] = ||y_p||^2 pulled down to x-row partitions
        ynp = psum.tile([32, 1], F32, tag="ynp")
        nc.tensor.matmul(
            ynp, lhsT=sel[:, _S0 : _S0 + 32], rhs=xn, start=True, stop=True
        )

        # args[:, 0:8] = D*d2(x_j, x_j') ; args[:, 8] = D*d2(x_j, y_j)
        argst = small.tile([32, 8], F32, tag="argst")
        nc.vector.tensor_scalar(
            out=argst, in0=xcp, scalar1=xn[0:32, :], scalar2=None, op0=add
        )
        args = small.tile([32, 9], F32, tag="args")
        nc.vector.tensor_tensor(out=args[:, 0:8], in0=argst, in1=cm2, op=add)
        t1 = small.tile([32, 1], F32, tag="t1")
        nc.vector.tensor_scalar(
            out=t1, in0=ynp, scalar1=xn[0:32, :], scalar2=None, op0=add
        )
        nc.vector.tensor_scalar(
            out=args[:, 8:9], in0=xy, scalar1=-2.0, scalar2=t1, op0=mult, op1=add
        )

        # exp(-beta*d2): scale folds in -beta/D; accum_out sums the 8 pair cols
        e8 = small.tile([32, 8], F32, tag="e8")
        nc.scalar.activation(
            out=e8, in_=args[:, 0:8], func=EXP, scale=bvec,
            accum_out=sc[0:32, 0:1],
        )
        nc.scalar.activation(
            out=sc[0:32, 1:2], in_=args[:, 8:9], func=EXP, scale=bvec
        )

        # per-sample sums over the 8 population rows
        psm = psum.tile([NS, 2], F32, tag="psm")
        nc.tensor.matmul(psm, lhsT=sel[:, _P0 : _P0 + NS], rhs=sc, start=True, stop=True)

        # finals: [score, conf, inter, inter_mult]
        fin = small.tile([NS, 4], F32, tag="fin")
        nc.vector.tensor_scalar(
            out=fin[:, 1:2], in0=psm[:, 1:2], scalar1=1.0 / M, scalar2=None, op0=mult
        )
        npair = float(M * (M - 1))
        nc.vector.tensor_scalar(
            out=fin[:, 2:3], in0=psm[:, 0:1],
            scalar1=1.0 / npair, scalar2=-M / npair, op0=mult, op1=add,
        )
        half_lam = LAMBDA_VAL / 2.0
        nc.vector.tensor_scalar(
            out=fin[:, 3:4], in0=psm[:, 0:1],
            scalar1=half_lam / npair, scalar2=-M * half_lam / npair,
            op0=mult, op1=add,
        )
        nc.vector.tensor_tensor(
            out=fin[:, 0:1], in0=fin[:, 3:4], in1=fin[:, 1:2], op=sub
        )
        nc.sync.dma_start(out=res_d[:], in_=fin)

    nc.compile()
    return nc


_PROG = {}
_CONSTS = None


def _get_prog(cast_mode=CAST_MODE):
    if cast_mode not in _PROG:
        _PROG[cast_mode] = _build_program(cast_mode)
    return _PROG[cast_mode]


def _make_in_maps(x, y, t, cast_mode=CAST_MODE):
    global _CONSTS
    if _CONSTS is None:
        _CONSTS = _build_consts()
    sel, i64 = _CONSTS
    if cast_mode == "bf16":
        import ml_dtypes

        zt_np_dt = ml_dtypes.bfloat16
    else:
        zt_np_dt = np.float32
    in_maps = []
    for c in range(NCORES):
        xc = x[c * NS : (c + 1) * NS].reshape(NS * M, D)
        yc = y[c * NS : (c + 1) * NS].reshape(NS * M, D)
        z = np.concatenate([xc, yc], axis=0)  # [64, D]
        # feature-major: zt[p, k*64 + r] = z[r, k*128 + p]
        zt = np.ascontiguousarray(
            z.reshape(R, NCH, 128).transpose(2, 1, 0).reshape(128, FREE),
            dtype=zt_np_dt,
        )
        in_maps.append(
            {
                "zt": zt,
                "tq": np.ascontiguousarray(
                    t[c * NS : (c + 1) * NS].reshape(NS, 1), dtype=np.int32
                ),
                "sel": sel,
                "i64": i64,
            }
        )
    return in_maps


def _run(x, y, t, trace=False, cast_mode=CAST_MODE, **spmd_kwargs):
    x = np.asarray(x, dtype=np.float32)
    y = np.asarray(y, dtype=np.float32)
    t = np.asarray(t, dtype=np.int32)
    nc = _get_prog(cast_mode)
    in_maps = _make_in_maps(x, y, t, cast_mode)
    br = run_bass_kernel_spmd(
        nc, in_maps, list(range(NCORES)), trace=trace, **spmd_kwargs
    )
    out = np.concatenate(
        [np.asarray(r["res"], dtype=np.float32) for r in br.results], axis=0
    )  # [32, 4]
    outs = tuple(np.ascontiguousarray(out[:, i]) for i in range(4))
    return outs, br


def kernel(x, y, t):
    """(score, confinement, interaction, interaction_mult), each [32] f32."""
    outs, _ = _run(x, y, t)
    return outs



# revision 2
# speedup vs baseline: 1.3710x; 1.3710x over previous
"""Trainium2 Bass kernel for nn_GeneralizedKernelScore (loss_fn).

Math per sample n (M=8 population members, D=12288 features):
    beta      = 2.0 - 1.9*t/999                      (linear schedule from t)
    conf[n]   = mean_j    exp(-beta*||x_j - y_j||^2 / D)
    inter[n]  = mean_{j!=j'} exp(-beta*||x_j - x_j'||^2 / D)
    im[n]     = inter/2
    score[n]  = im - conf

Strategy (data-parallel over batch, 4 samples per core on 8 cores):
Each core owns Z = [X; Y] (64 rows x 12288) in fp8-e4m3, pre-transposed
on the host to feature-major [128, 96*64] so the contraction dim lands
on SBUF partitions.  All distances come from the 64x64 Gram matrix
G = Z Z^T, computed as 96 accumulating matmuls over 128-wide feature
slices.  Adjacent slices are paired into one 128-col weight load and
issued as two concurrent column-group matmuls (tile cols 0-63 -> PSUM
partitions 0-63, cols 64-127 -> partitions 64-127), so G arrives as
two partition-stacked halves Ga/Gb that every later linear step
processes full-width.

Epilogue (5 cross-engine hops):
  DVE   : masked reduces on [Ga;Gb] -> split diag norms xn2[128,1],
          per-sample pair blocks cm[128,8], x.y diag xy[128,1]
  PE    : three matmuls accumulate into pt[32,9]; the [128->32]
          partition fold of the split halves rides the contraction:
            pt += W2^T (mask9 . xn2)    (norms spread across cols +
                                         y-norm into col 8)
            pt += W3^T [cm | xy]        (-2 G terms, halves folded)
            pt += (W3mask . xn2)^T ones (row-norm broadcast to all cols)
          giving pt[p,f] = D*d2(x_p, x_{s,f}) for f != p%8,
                 pt[p,p%8] = 0, pt[p,8] = D*d2(x_p, y_p)
  ACT   : one Exp with per-partition scale -beta/D (host-computed from
          t), accum_out = row sums
  PE    : two tiny matmuls contract the 8 rows of each sample
  DVE+DMA: copy [4,2] out; host applies the constant affine to get
          (score, confinement, interaction, interaction_mult).

DMA: input split in 4 chunks issued alternately on the two HWDGE
queues (SP + Activation) so the rings drain in parallel; constants
ride a 5th transfer.
"""

from contextlib import ExitStack

import numpy as np
import ml_dtypes

import concourse.bass as bass
import concourse.mybir as mybir
import concourse.tile as tile
from concourse import bacc
from concourse.bass_utils import run_bass_kernel_spmd

# problem shape (hardcoded per spec)
N, M, D = 32, 8, 12288
NUM_TIMESTEPS = 1000
BETA_START, BETA_END = 2.0, 0.1
LAMBDA_VAL = 1.0

NCORES = 8
NS = N // NCORES          # 4 samples per core
R = 2 * NS * M            # 64 Z-rows per core (32 x-rows then 32 y-rows)
NCH = D // 128            # 96 contraction chunks of the feature dim
FREE = NCH * R            # 6144 free columns of Z^T
NDMA = 4                  # input DMA chunks
CHF = FREE // NDMA        # 1536 cols per DMA chunk
CHP = NCH // NDMA // 2    # 12 ldw-pairs per DMA chunk

# const tensor column layout
_I64, _M2, _MXY, _MK9, _W2, _W3, _ON9, _P4, _BV = (
    0, 64, 128, 192, 201, 233, 265, 274, 278,
)
CONW = 279

F32 = mybir.dt.float32
FP8 = mybir.dt.float8e4
NP_FP8 = ml_dtypes.float8_e4m3


def _build_consts():
    k = np.arange(128)[:, None]
    km = k % 64
    c64 = np.arange(64)[None, :]
    # I64d2: diag mask per half: (c == k%64)
    i64d2 = (c64 == km).astype(np.float32)
    # M2full: -2 on same-sample x-x block, x-rows of each half only
    m2 = np.where((km < 32) & (c64 < 32) & (c64 // 8 == km // 8), -2.0, 0.0)
    # Mxy: -2 on the x.y diagonal element (col 32 + row), x-rows only
    mxy = np.where((km < 32) & (c64 == 32 + km), -2.0, 0.0)
    # mask9: cols 0-7 route x-row norms by j = k%8; col 8 flags y-rows
    f9 = np.arange(9)[None, :]
    mk9 = np.where(
        (f9 < 8) & (km < 32) & (k % 8 == f9), 1.0,
        np.where((f9 == 8) & (km >= 32), 1.0, 0.0),
    )
    m32 = np.arange(32)[None, :]
    # W2: A-part selects same-sample x-rows; B-part selects the y-row
    w2 = (((km < 32) & (km // 8 == m32 // 8)) | (km == 32 + m32)).astype(
        np.float32
    )
    # W3: fold [128]->[32] partitions (k%64 == m, x-rows only)
    w3 = ((km == m32) & (km < 32)).astype(np.float32)
    on9 = np.ones((128, 9), dtype=np.float32)
    # P4: sample selector (k//8 == s) on partitions 0-31
    p4 = ((k < 32) & (k // 8 == np.arange(4)[None, :])).astype(np.float32)
    bv = np.zeros((128, 1), dtype=np.float32)  # filled per-core with -beta/D
    con = np.concatenate(
        [i64d2, m2, mxy, mk9, w2, w3, on9, p4, bv], axis=1
    ).astype(np.float32)
    assert con.shape == (128, CONW)
    return con


def _build_program():
    nc = bacc.Bacc("TRN2", target_bir_lowering=False)
    zt = nc.dram_tensor("zt", [128, FREE], FP8, kind="ExternalInput")
    con_d = nc.dram_tensor("con", [128, CONW], F32, kind="ExternalInput")
    res_d = nc.dram_tensor("res", [NS, 2], F32, kind="ExternalOutput")

    mult, add = mybir.AluOpType.mult, mybir.AluOpType.add
    EXP = mybir.ActivationFunctionType.Exp

    with ExitStack() as ctx:
        tc = ctx.enter_context(tile.TileContext(nc))
        small = ctx.enter_context(tc.tile_pool(name="small", bufs=1))
        zbf_p = ctx.enter_context(tc.tile_pool(name="zbf", bufs=NDMA))
        psum = ctx.enter_context(tc.tile_pool(name="psum", bufs=1, space="PSUM"))

        # --- input + const DMAs, alternating the two HWDGE queues -----
        zbf = []
        for i in range(NDMA):
            zc = zbf_p.tile([128, CHF], FP8, tag="zbf")
            eng = nc.sync if i % 2 == 0 else nc.scalar
            eng.dma_start(out=zc, in_=zt[:, i * CHF : (i + 1) * CHF])
            zbf.append(zc)
        con = small.tile([128, CONW], F32, tag="con")
        nc.sync.dma_start(out=con, in_=con_d[:])

        # preload the Exp LUT while DMAs run
        warm = small.tile([1, 1], F32, tag="warm")
        nc.vector.memset(warm, 0.0)
        nc.scalar.activation(out=warm, in_=warm, func=EXP)

        # --- Gram: G = Z Z^T as two column-group halves ----------------
        # pair weights [chunk_j | chunk_j+1]; cg (0,0) accumulates even
        # chunks into PSUM partitions 0-63, cg (0,64) odd chunks into
        # 64-127.  Both matmuls of a pair stream concurrently.
        G = psum.tile([128, R], F32, tag="G")
        npair = NCH // 2
        for i in range(NDMA):
            for j in range(CHP):
                p = i * CHP + j
                a = zbf[i][:, (2 * j) * R : (2 * j + 1) * R]
                b = zbf[i][:, (2 * j + 1) * R : (2 * j + 2) * R]
                nc.tensor.matmul(
                    G[0:64, :], lhsT=a, rhs=a,
                    start=(p == 0), stop=(p == npair - 1),
                    skip_group_check=True,
                )
                nc.tensor.matmul(
                    G[64:128, :], lhsT=b, rhs=b,
                    start=(p == 0), stop=(p == npair - 1),
                    skip_group_check=True,
                )

        # --- epilogue ---------------------------------------------------
        # [V] masked reduces, full-width over both stacked halves
        s128 = small.tile([128, R], F32, tag="s128")
        nc.vector.tensor_tensor(
            out=s128, in0=G, in1=con[:, _I64 : _I64 + 64], op=mult
        )
        xn2 = small.tile([128, 1], F32, tag="xn2")
        nc.vector.reduce_sum(out=xn2, in_=s128, axis=mybir.AxisListType.X)
        rhs9 = small.tile([128, 9], F32, tag="rhs9")
        nc.vector.tensor_scalar(
            out=rhs9, in0=con[:, _MK9 : _MK9 + 9], scalar1=xn2, scalar2=None,
            op0=mult,
        )
        xnw = small.tile([128, 32], F32, tag="xnw")
        nc.vector.tensor_scalar(
            out=xnw, in0=con[:, _W3 : _W3 + 32], scalar1=xn2, scalar2=None,
            op0=mult,
        )
        gm = small.tile([128, R], F32, tag="gm")
        nc.vector.tensor_tensor(
            out=gm, in0=G, in1=con[:, _M2 : _M2 + 64], op=mult
        )
        scr = small.tile([128, 9], F32, tag="scr")
        nc.vector.reduce_sum(
            out=scr[:, 0:8],
            in_=gm.rearrange("p (g f) -> p f g", g=8),
            axis=mybir.AxisListType.X,
        )
        sxy = small.tile([128, R], F32, tag="sxy")
        nc.vector.tensor_tensor(
            out=sxy, in0=G, in1=con[:, _MXY : _MXY + 64], op=mult
        )
        nc.vector.reduce_sum(
            out=scr[:, 8:9], in_=sxy, axis=mybir.AxisListType.X
        )

        # [T] pt = norms-spread + (-2G terms, halves folded) + row-norm
        pt = psum.tile([32, 9], F32, tag="pt")
        nc.tensor.matmul(
            pt, lhsT=con[:, _W2 : _W2 + 32], rhs=rhs9, start=True, stop=False
        )
        nc.tensor.matmul(
            pt, lhsT=con[:, _W3 : _W3 + 32], rhs=scr, start=False, stop=False
        )
        nc.tensor.matmul(
            pt, lhsT=xnw, rhs=con[:, _ON9 : _ON9 + 9], start=False, stop=True
        )

        # [S] exp(-beta/D * pt): pairs in cols 0-7 (diag slot -> 1),
        # confinement in col 8; accum_out sums each row
        e9 = small.tile([32, 9], F32, tag="e9")
        sc = small.tile([32, 1], F32, tag="sc")
        nc.scalar.activation(
            out=e9, in_=pt, func=EXP, scale=con[0:32, _BV : _BV + 1],
            accum_out=sc,
        )

        # [T] per-sample sums over the 8 population rows
        pc = psum.tile([NS, 2], F32, tag="pc")
        nc.tensor.matmul(
            pc[:, 0:1], lhsT=con[0:32, _P4 : _P4 + 4], rhs=sc,
            start=True, stop=True, skip_group_check=True,
        )
        nc.tensor.matmul(
            pc[:, 1:2], lhsT=con[0:32, _P4 : _P4 + 4], rhs=e9[:, 8:9],
            start=True, stop=True, skip_group_check=True,
        )

        # [V] -> DMA out
        fin = small.tile([NS, 2], F32, tag="fin")
        nc.vector.tensor_copy(out=fin, in_=pc)
        nc.sync.dma_start(out=res_d[:], in_=fin)

    nc.compile()
    return nc


_PROG = None
_CONSTS = None


def _get_prog():
    global _PROG
    if _PROG is None:
        _PROG = _build_program()
    return _PROG


def _make_in_maps(x, y, t):
    global _CONSTS
    if _CONSTS is None:
        _CONSTS = _build_consts()
    beta = BETA_START + (BETA_END - BETA_START) * (
        t.astype(np.float64) / (NUM_TIMESTEPS - 1)
    )
    in_maps = []
    for c in range(NCORES):
        xc = x[c * NS : (c + 1) * NS].reshape(NS * M, D)
        yc = y[c * NS : (c + 1) * NS].reshape(NS * M, D)
        z = np.concatenate([xc, yc], axis=0)  # [64, D]
        # feature-major: zt[p, k*64 + r] = z[r, k*128 + p]
        zt = np.ascontiguousarray(
            z.reshape(R, NCH, 128).transpose(2, 1, 0).reshape(128, FREE)
        ).astype(NP_FP8)
        con = _CONSTS.copy()
        bcore = np.repeat(beta[c * NS : (c + 1) * NS], M)  # [32]
        con[0:32, _BV] = (-bcore / D).astype(np.float32)
        in_maps.append({"zt": zt, "con": con})
    return in_maps


def _run(x, y, t, trace=False, **spmd_kwargs):
    x = np.asarray(x, dtype=np.float32)
    y = np.asarray(y, dtype=np.float32)
    t = np.asarray(t, dtype=np.int32)
    nc = _get_prog()
    in_maps = _make_in_maps(x, y, t)
    br = run_bass_kernel_spmd(
        nc, in_maps, list(range(NCORES)), trace=trace, **spmd_kwargs
    )
    S = np.concatenate(
        [np.asarray(r["res"], dtype=np.float32) for r in br.results], axis=0
    )  # [32, 2]: S0 = pairs + 8 + conf_sum, S1 = conf_sum
    conf = S[:, 1] / M
    inter = (S[:, 0] - S[:, 1] - M) / (M * (M - 1))
    im = (LAMBDA_VAL / 2.0) * inter
    score = im - conf
    outs = tuple(
        np.ascontiguousarray(v, dtype=np.float32)
        for v in (score, conf, inter, im)
    )
    return outs, br


def kernel(x, y, t):
    """(score, confinement, interaction, interaction_mult), each [32] f32."""
    outs, _ = _run(x, y, t)
    return outs
